# revision 1
# baseline (speedup 1.0000x reference)
"""Trainium2 Bass kernel for nn_AttentionModule (sparse_attention).

Strategy (8 NeuronCores, no collectives):
  core c -> batch b = c // 2, query-half th = c % 2 (T' = 512 queries).
  Each core computes, for its (b, th):
    qT  [A,T'] = Wq^T @ x_slice (+bq, LARoPE)          (x is already [D,T])
    kT  [A,L]  = Wk^T @ ctx^T   (+bk, LARoPE)          (ctx^T prepared on host)
    v'  [L,A'] = ctx @ Wv (+bv), with a ones-column per head (A' = 16*65)
    attnT[h]  [L,T'] = exp((k_h q_h^T)/32 + mask_bias)  (softmax numerator,
                key-masked via per-partition bias; logits are tiny so no
                max-subtraction is needed)
    O'_h [65,T'] = v'_h^T @ attnT_h   (row 64 = softmax denominator)
    Osb  [A,T'] = O'_h[:64] * broadcast(x_mask / denom)
    y    [D,T'] = Wo^T @ Osb + bo x xm                  (= final out^T slice)
  Host assembles y slices into out[B, D, T].

All matmuls run in bf16 (fp32 PSUM accumulation).

Measured on trn2 (8 cores concurrent, NTFF profile of core 0):
  HW exec time 248518 ns/core, relative error vs the fp32 reference
  3.73e-3 (absmax/scale; rel_l2 2.9e-3) — pure bf16 rounding noise.
"""

import contextlib
import math
import os
import sys

import numpy as np


def _ensure_paths():
    for p in ("/opt/trn_rl_repo", "/root/.axon_site/_ro/trn_rl_repo"):
        if os.path.isdir(p) and p not in sys.path:
            sys.path.insert(0, p)


try:
    import concourse.bass as bass  # noqa: F401
except ImportError:
    _ensure_paths()

import ml_dtypes
import concourse.bass as bass
import concourse.tile as tile
from concourse import bacc, bass2jax, mybir

# Problem shapes (hardcoded per the module definition).
B = 4
T = 1024
L = 1024
DM = 1024  # d_model
AD = 1024  # attn_dim
H = 16
HD = 64  # head dim
TQ = T // 2  # per-core query slice
N_CORES = 8
SCALE = 1.0 / math.sqrt(AD)  # note: module scales by sqrt(attn_dim)
ROPE_GAMMA = 10.0
ROTARY_BASE = 10000.0
MASK_BIAS = -30000.0  # exp(x + MASK_BIAS) underflows to exactly 0.0 in fp32

MDT = mybir.dt.bfloat16
NP_MDT = ml_dtypes.bfloat16
F32 = mybir.dt.float32

AL = mybir.AluOpType
AF = mybir.ActivationFunctionType


def build_program():
    nc = bacc.Bacc("TRN2", target_bir_lowering=False, debug=False)

    def din(name, shape, dt):
        return nc.dram_tensor(name, shape, dt, kind="ExternalInput").ap()

    xs = din("xs", [DM, TQ], MDT)          # x[b][:, tslice]
    ctxT = din("ctxT", [DM, L], MDT)       # context[b].T
    wq = din("wq", [DM, AD], MDT)
    wk = din("wk", [DM, AD], MDT)
    wv = din("wv", [DM, AD], MDT)
    wo = din("wo", [AD, DM], MDT)
    bqc = din("bqc", [128, 8], F32)        # bq[a*128+p] at [p, a]
    bkc = din("bkc", [128, 8], F32)
    bvr = din("bvr", [1, AD], MDT)
    bor = din("bor", [1, DM], MDT)
    onesr = din("onesr", [1, 128], MDT)
    cosq = din("cosq", [128, TQ], F32)
    sin2q = din("sin2q", [128, TQ], F32)
    cosk = din("cosk", [128, L], F32)
    sin2k = din("sin2k", [128, L], F32)
    cmb = din("cmb", [128, 8], F32)        # key-mask bias per (p, l_tile)
    xmf = din("xmf", [1, TQ], F32)         # query mask (f32)
    xmm = din("xmm", [1, TQ], MDT)         # query mask (bf16, for bo (x) xm)
    perm = din("perm", [128, 128], MDT)    # partition permutation p -> p^32
    y = nc.dram_tensor("y", [DM, TQ], F32, kind="ExternalOutput").ap()

    with tile.TileContext(nc) as tc, contextlib.ExitStack() as ctx:
        sb = ctx.enter_context(tc.tile_pool(name="sb", bufs=1))
        ps = ctx.enter_context(tc.tile_pool(name="ps", bufs=2, space="PSUM"))

        # ---- constants -------------------------------------------------
        C = {}
        for nm, ap in [
            ("bqc", bqc), ("bkc", bkc), ("bvr", bvr), ("bor", bor),
            ("onesr", onesr), ("cosq", cosq), ("sin2q", sin2q),
            ("cosk", cosk), ("sin2k", sin2k), ("cmb", cmb),
            ("xmf", xmf), ("xmm", xmm), ("perm", perm),
        ]:
            t = sb.tile(list(ap.shape), ap.dtype, tag=nm, name=f"c_{nm}", bufs=1)
            nc.sync.dma_start(t[:], ap)
            C[nm] = t

        # ---- activations / weights ------------------------------------
        xs_t = []
        xs_r = xs.rearrange("(n p) t -> n p t", p=128)
        for d in range(8):
            t = sb.tile([128, TQ], MDT, tag="xs", bufs=8, name=f"xs{d}")
            nc.sync.dma_start(t[:], xs_r[d])
            xs_t.append(t)

        def load_w(ap, nm):
            ts_ = []
            r = ap.rearrange("(n p) c -> n p c", p=128)
            for d in range(8):
                t = sb.tile([128, 1024], MDT, tag="w", bufs=24, name=f"{nm}{d}")
                nc.sync.dma_start(t[:], r[d])
                ts_.append(t)
            return ts_

        wq_t = load_w(wq, "wq")
        wk_t = load_w(wk, "wk")
        wv_t = load_w(wv, "wv")

        ctx_t = []
        ctx_r = ctxT.rearrange("(n p) l -> n p l", p=128)
        for d in range(8):
            t = sb.tile([128, L], MDT, tag="ctx", bufs=8, name=f"ctx{d}")
            nc.sync.dma_start(t[:], ctx_r[d])
            ctx_t.append(t)

        # ---- Q phase: qT[a] = rope(Wq^T @ x + bq) ----------------------
        # The perm-matmul of unit a is emitted after unit a+1's projection
        # so the PE never waits on the DVE stt results.
        qT_t = [None] * 8
        pend_q = []

        def flush_q():
            a, wsb, asb = pend_q.pop(0)
            pw_ps = ps.tile([128, TQ], F32, tag="pp", bufs=4, name=f"qpw{a}")
            nc.tensor.matmul(pw_ps[:], C["perm"][:], wsb[:], start=True, stop=True)
            qt = sb.tile([128, TQ], MDT, tag="qT", bufs=8, name=f"qT{a}")
            nc.vector.tensor_add(qt[:], pw_ps[:], asb[:])
            qT_t[a] = qt

        for a in range(8):
            q_ps = ps.tile([128, TQ], F32, tag="pp", bufs=4, name=f"qps{a}")
            for d in range(8):
                nc.tensor.matmul(
                    q_ps[:], wq_t[d][:, a * 128:(a + 1) * 128], xs_t[d][:],
                    start=(d == 0), stop=(d == 7),
                )
            wsb = sb.tile([128, TQ], MDT, tag="ropeW", bufs=4, name=f"qw{a}")
            nc.vector.scalar_tensor_tensor(
                wsb[:], q_ps[:], C["bqc"][:, a:a + 1], C["sin2q"][:],
                op0=AL.add, op1=AL.mult,
            )
            asb = sb.tile([128, TQ], MDT, tag="ropeA", bufs=4, name=f"qa{a}")
            nc.vector.scalar_tensor_tensor(
                asb[:], q_ps[:], C["bqc"][:, a:a + 1], C["cosq"][:],
                op0=AL.add, op1=AL.mult,
            )
            pend_q.append((a, wsb, asb))
            if len(pend_q) > 1:
                flush_q()
        while pend_q:
            flush_q()

        # ---- V phase: v'[l] = (ctx @ Wv + bv | ones) -------------------
        vP_t = []
        for l in range(8):
            vt = sb.tile([128, 1040], MDT, tag="vP", bufs=8, name=f"vP{l}")
            for ah in range(2):
                v_ps = ps.tile([128, 512], F32, tag="pp", bufs=4, name=f"vps{l}_{ah}")
                for d in range(8):
                    nc.tensor.matmul(
                        v_ps[:], ctx_t[d][:, l * 128:(l + 1) * 128],
                        wv_t[d][:, ah * 512:(ah + 1) * 512],
                        start=(d == 0), stop=False,
                    )
                nc.tensor.matmul(
                    v_ps[:], C["onesr"][0:1, 0:128],
                    C["bvr"][0:1, ah * 512:(ah + 1) * 512],
                    start=False, stop=True,
                )
                out_ap = vt[:, ah * 520:(ah + 1) * 520].rearrange(
                    "p (h e) -> p h e", e=65)[:, :, 0:64]
                in_ap = v_ps[:].rearrange("p (h d) -> p h d", d=64)
                nc.scalar.copy(out_ap, in_ap)  # ACT; keeps DVE free
            ones_ap = vt[:, :].rearrange("p (h e) -> p h e", e=65)[:, :, 64:65]
            nc.gpsimd.memset(ones_ap, 1.0)
            vP_t.append(vt)

        wo_t = load_w(wo, "wo")  # reuses wq slots once the Q phase retires

        # ---- gangs: per head-pair g: k-proj, QK+exp, PV(g-1) -----------
        kT_t = [None] * 8
        attn_t = [[None] * 8 for _ in range(8)]
        osb_t = [None] * 8

        kpend = {}

        def kproj_half(g, lh):
            if lh == 0:
                kT_t[g] = sb.tile([128, L], MDT, tag="kT", bufs=8, name=f"kT{g}")
                kpend[g] = []
            sl = slice(lh * 512, (lh + 1) * 512)
            k_ps = ps.tile([128, 512], F32, tag="pp", bufs=4, name=f"kps{g}_{lh}")
            for d in range(8):
                nc.tensor.matmul(
                    k_ps[:], wk_t[d][:, g * 128:(g + 1) * 128],
                    ctx_t[d][:, sl], start=(d == 0), stop=(d == 7),
                )
            wsb = sb.tile([128, 512], MDT, tag="ropeW", bufs=4, name=f"kw{g}_{lh}")
            nc.vector.scalar_tensor_tensor(
                wsb[:], k_ps[:], C["bkc"][:, g:g + 1], C["sin2k"][:, sl],
                op0=AL.add, op1=AL.mult,
            )
            asb = sb.tile([128, 512], MDT, tag="ropeA", bufs=4, name=f"ka{g}_{lh}")
            nc.vector.scalar_tensor_tensor(
                asb[:], k_ps[:], C["bkc"][:, g:g + 1], C["cosk"][:, sl],
                op0=AL.add, op1=AL.mult,
            )
            kpend[g].append((lh, wsb, asb))

        def kproj_flush(g):
            lh, wsb, asb = kpend[g].pop(0)
            sl = slice(lh * 512, (lh + 1) * 512)
            pw_ps = ps.tile([128, 512], F32, tag="pp", bufs=4, name=f"kpw{g}_{lh}")
            nc.tensor.matmul(pw_ps[:], C["perm"][:], wsb[:], start=True, stop=True)
            nc.vector.tensor_add(kT_t[g][:, sl], pw_ps[:], asb[:])

        def emit_qk_unit(g, l):
            qk_ps = ps.tile([128, 1024], F32, tag="qk", bufs=2, name=f"qk{g}_{l}")
            for h2 in range(2):
                nc.tensor.matmul(
                    qk_ps[:, h2 * 512:(h2 + 1) * 512],
                    kT_t[g][h2 * 64:(h2 + 1) * 64, l * 128:(l + 1) * 128],
                    qT_t[g][h2 * 64:(h2 + 1) * 64, :],
                    start=True, stop=True,
                )
            at = sb.tile([128, 1024], MDT, tag="attn", bufs=16, name=f"at{g}_{l}")
            nc.scalar.activation(
                at[:], qk_ps[:], AF.Exp, bias=C["cmb"][:, l:l + 1], scale=SCALE,
            )
            attn_t[g][l] = at

        def emit_pv(g, h2, lrange, o_ps):
            h = 2 * g + h2
            for l in lrange:
                nc.tensor.matmul(
                    o_ps[:], vP_t[l][:, h * 65:h * 65 + 65],
                    attn_t[g][l][:, h2 * 512:(h2 + 1) * 512],
                    start=(l == 0), stop=(l == 7),
                )

        def emit_norm(g, h2, o_ps, osb):
            rc = sb.tile([1, TQ], F32, tag="recip", bufs=2, name=f"rc{g}_{h2}")
            nc.vector.reciprocal(rc[:], o_ps[64:65, :])
            rx = sb.tile([1, TQ], F32, tag="rxm", bufs=2, name=f"rx{g}_{h2}")
            nc.vector.tensor_mul(rx[:], rc[:], C["xmf"][:])
            bc = sb.tile([64, TQ], F32, tag="bc", bufs=2, name=f"bc{g}_{h2}")
            nc.gpsimd.partition_broadcast(bc[:], rx[:], channels=64)
            nc.vector.tensor_mul(
                osb[h2 * 64:(h2 + 1) * 64, :], o_ps[0:64, :], bc[:],
            )

        def emit_pv_gang(gp):
            osb = sb.tile([128, TQ], MDT, tag="osb", bufs=8, name=f"osb{gp}")
            osb_t[gp] = osb
            o0 = ps.tile([65, TQ], F32, tag="pp", bufs=4, name=f"o{gp}_0")
            o1 = ps.tile([65, TQ], F32, tag="pp", bufs=4, name=f"o{gp}_1")
            return osb, o0, o1

        # Two-deep software pipeline: iteration i runs kproj(i) dense on
        # the PE while QK(i-1) units trickle against ACT's exp and
        # PV(i-2) consumes the previous gang — QK units are spread
        # between dense PE chunks so the PE never waits out a full exp.
        for i in range(10):
            chunks = []
            if i < 8:
                chunks.append(lambda g=i: kproj_half(g, 0))
                chunks.append(lambda g=i: (kproj_half(g, 1), kproj_flush(g)))
                chunks.append(lambda g=i: kproj_flush(g))
            if i >= 2:
                gp2 = i - 2
                osb = sb.tile([128, TQ], MDT, tag="osb", bufs=8,
                              name=f"osb{gp2}")
                osb_t[gp2] = osb
                box = {}

                # o0/o1 are allocated lazily at first use so their psum
                # slot requests sit AFTER kproj(i)'s in program order —
                # otherwise the kproj stalls behind the previous gang's
                # norm tail (measured ~5.6us/gang on HW)
                def c2(g=gp2, b=box):
                    b["o0"] = ps.tile([65, TQ], F32, tag="pp", bufs=4,
                                      name=f"o{g}_0")
                    emit_pv(g, 0, range(0, 4), b["o0"])

                def c3(g=gp2, b=box):
                    emit_pv(g, 0, range(4, 8), b["o0"])

                def c4(g=gp2, b=box, s=osb):
                    emit_norm(g, 0, b["o0"], s)
                    b["o1"] = ps.tile([65, TQ], F32, tag="pp", bufs=4,
                                      name=f"o{g}_1")
                    emit_pv(g, 1, range(0, 4), b["o1"])

                def c5(g=gp2, b=box, s=osb):
                    emit_pv(g, 1, range(4, 8), b["o1"])
                    emit_norm(g, 1, b["o1"], s)

                chunks.extend([c2, c3, c4, c5])
            qks = ([lambda g=i - 1, l=l: emit_qk_unit(g, l) for l in range(8)]
                   if 1 <= i <= 8 else [])
            for j in range(max(len(qks), len(chunks))):
                if j < len(qks):
                    qks[j]()
                if j < len(chunks):
                    chunks[j]()

        # ---- out-proj: y[d] = Wo^T @ Osb + bo (x) xm -------------------
        y_r = y.rearrange("(n p) t -> n p t", p=128)
        for d in range(8):
            o_ps = ps.tile([128, TQ], F32, tag="pp", bufs=4, name=f"ops{d}")
            for a in range(8):
                nc.tensor.matmul(
                    o_ps[:], wo_t[a][:, d * 128:(d + 1) * 128], osb_t[a][:],
                    start=(a == 0), stop=False,
                )
            nc.tensor.matmul(
                o_ps[:], C["bor"][0:1, d * 128:(d + 1) * 128], C["xmm"][0:1, :],
                start=False, stop=True,
            )
            yt = sb.tile([128, TQ], F32, tag="outsb", bufs=4, name=f"yt{d}")
            nc.scalar.copy(yt[:], o_ps[:])  # ACT; keeps DVE free
            nc.sync.dma_start(y_r[d], yt[:])

    nc.compile()
    return nc


_PROGRAM = None


def _get_program():
    global _PROGRAM
    if _PROGRAM is None:
        _PROGRAM = build_program()
    return _PROGRAM


def _host_prep(x, context, x_mask, context_mask, Wq, bq, Wk, bk, Wv, bv, Wo, bo):
    """Build the 8 per-core input maps."""
    f32 = np.float32
    x = np.asarray(x, f32)
    context = np.asarray(context, f32)
    xm = np.asarray(x_mask).astype(f32)      # [B,1,T]
    cm = np.asarray(context_mask).astype(f32)  # [B,1,L]

    len_q = xm.sum(axis=(1, 2))  # [B]
    len_k = cm.sum(axis=(1, 2))

    inv_freq = 1.0 / (ROTARY_BASE ** (np.arange(0, HD, 2, dtype=f32) / HD))
    theta = (inv_freq * ROPE_GAMMA).astype(f32)  # [32]

    p = np.arange(128)
    pm32 = p % 32
    sgn_sin2 = np.where((p % 64) < 32, 1.0, -1.0).astype(f32)[:, None]

    perm = np.zeros((128, 128), f32)
    perm[p, p ^ 32] = 1.0  # lhsT: out[m] = sum_k perm[k, m] * in[k] = in[m^32]

    shared = {
        "wq": np.ascontiguousarray(Wq).astype(NP_MDT),
        "wk": np.ascontiguousarray(Wk).astype(NP_MDT),
        "wv": np.ascontiguousarray(Wv).astype(NP_MDT),
        "wo": np.ascontiguousarray(Wo).astype(NP_MDT),
        "bqc": np.asarray(bq, f32).reshape(8, 128).T.copy(),
        "bkc": np.asarray(bk, f32).reshape(8, 128).T.copy(),
        "bvr": np.asarray(bv, f32).reshape(1, AD).astype(NP_MDT),
        "bor": np.asarray(bo, f32).reshape(1, DM).astype(NP_MDT),
        "onesr": np.ones((1, 128), NP_MDT),
        "perm": perm.astype(NP_MDT),
    }

    in_maps = []
    for c in range(N_CORES):
        b, th = c // 2, c % 2
        tsl = slice(th * TQ, (th + 1) * TQ)

        pos_q = (np.arange(T, dtype=f32)[tsl] / len_q[b])  # [TQ]
        fr_q = pos_q[None, :] * theta[pm32][:, None]       # [128, TQ]
        pos_k = np.arange(L, dtype=f32) / len_k[b]
        fr_k = pos_k[None, :] * theta[pm32][:, None]       # [128, L]

        m = dict(shared)
        m["xs"] = np.ascontiguousarray(x[b][:, tsl]).astype(NP_MDT)
        m["ctxT"] = np.ascontiguousarray(context[b].T).astype(NP_MDT)
        m["cosq"] = np.cos(fr_q).astype(f32)
        m["sin2q"] = (np.sin(fr_q) * sgn_sin2).astype(f32)
        m["cosk"] = np.cos(fr_k).astype(f32)
        m["sin2k"] = (np.sin(fr_k) * sgn_sin2).astype(f32)
        # 0.0 where the key is valid, MASK_BIAS where masked
        m["cmb"] = ((cm[b, 0] - 1.0) * (-MASK_BIAS)).reshape(8, 128).T.copy().astype(f32)
        m["xmf"] = xm[b, 0, tsl].reshape(1, TQ).astype(f32)
        m["xmm"] = xm[b, 0, tsl].reshape(1, TQ).astype(NP_MDT)
        in_maps.append(m)
    return in_maps


_JIT_CACHE = {}


def _run_concurrent(nc, in_maps, n_cores=N_CORES):
    """Run the same bass program on n_cores devices concurrently, one
    single-device PJRT execute per core.

    (run_bass_kernel_spmd's multi-core path uses an 8-device shard_map
    SPMD executable, which hangs under this axon terminal; per-device
    dispatch of the identical program is functionally equivalent for a
    collective-free kernel and works.)
    """
    import jax

    bass2jax.install_neuronx_cc_hook()
    key = id(nc)
    if key not in _JIT_CACHE:
        partition_name = (
            nc.partition_id_tensor.name if nc.partition_id_tensor else None
        )
        in_names, out_names, out_avals, zero_outs = [], [], [], []
        for alloc in nc.m.functions[0].allocations:
            if not isinstance(alloc, mybir.MemoryLocationSet):
                continue
            name = alloc.memorylocations[0].name
            if alloc.kind == "ExternalInput":
                if name != partition_name:
                    in_names.append(name)
            elif alloc.kind == "ExternalOutput":
                shape = tuple(alloc.tensor_shape)
                dtype = mybir.dt.np(alloc.dtype)
                out_names.append(name)
                out_avals.append(jax.core.ShapedArray(shape, dtype))
                zero_outs.append(np.zeros(shape, dtype))
        n_params = len(in_names)
        in_names_full = list(in_names) + list(out_names)
        if partition_name is not None:
            in_names_full.append(partition_name)
        in_names_full = tuple(in_names_full)

        def _body(*args):
            operands = list(args)
            if partition_name is not None:
                operands.append(bass2jax.partition_id_tensor())
            outs = bass2jax._bass_exec_p.bind(
                *operands, out_avals=tuple(out_avals), in_names=in_names_full,
                out_names=tuple(out_names), lowering_input_output_aliases=(),
                sim_require_finite=True, sim_require_nnan=True, nc=nc)
            return tuple(outs)

        donate_idx = tuple(range(n_params, n_params + len(out_names)))
        jfn = jax.jit(_body, donate_argnums=donate_idx, keep_unused=True)
        _JIT_CACHE[key] = (jfn, in_names, out_names, zero_outs)

    jfn, in_names, out_names, zero_outs = _JIT_CACHE[key]
    devices = jax.devices()[:n_cores]
    futs = []
    for c, dev in enumerate(devices):
        args = [jax.device_put(np.asarray(in_maps[c][nm]), dev) for nm in in_names]
        args += [jax.device_put(z, dev) for z in zero_outs]
        futs.append(jfn(*args))
    return [
        {nm: np.asarray(futs[c][i]) for i, nm in enumerate(out_names)}
        for c in range(n_cores)
    ]


def precompile():
    """AOT-compile the NEFF (client-side) without touching the data plane."""
    import jax

    nc = _get_program()
    _run_concurrent(nc, [], n_cores=0)  # populate _JIT_CACHE only
    jfn, in_names, out_names, zero_outs = _JIT_CACHE[id(nc)]
    specs = []
    for alloc in nc.m.functions[0].allocations:
        if not isinstance(alloc, mybir.MemoryLocationSet):
            continue
        name = alloc.memorylocations[0].name
        if alloc.kind == "ExternalInput" and name in in_names:
            specs.append((name, jax.ShapeDtypeStruct(
                tuple(alloc.tensor_shape), mybir.dt.np(alloc.dtype))))
    by_name = dict(specs)
    args = [by_name[nm] for nm in in_names]
    args += [jax.ShapeDtypeStruct(z.shape, z.dtype) for z in zero_outs]
    compiled = jfn.lower(*args).compile()
    return compiled


def kernel(x, context, x_mask, context_mask, Wq, bq, Wk, bk, Wv, bv, Wo, bo):
    nc = _get_program()
    in_maps = _host_prep(x, context, x_mask, context_mask,
                         Wq, bq, Wk, bk, Wv, bv, Wo, bo)
    results = _run_concurrent(nc, in_maps, N_CORES)

    out = np.empty((B, DM, T), np.float32)
    for c in range(N_CORES):
        b, th = c // 2, c % 2
        out[b][:, th * TQ:(th + 1) * TQ] = results[c]["y"]
    return out



# revision 14
# speedup vs baseline: 1.3754x; 1.3754x over previous
"""Trainium2 Bass kernel for nn_AttentionModule (sparse_attention).

Strategy v2 (8 NeuronCores, no collectives):
  core c -> batch b = c // 2, head-half hh = c % 2 (8 of 16 heads, all
  T=1024 queries).  Splitting heads instead of queries halves the K/V
  projection work (it was duplicated across the query-split pair) at the
  cost of a host-side pair-sum of the two partial out-projections.

  Each core computes, for its (b, hh) attn-dim slice A' = 512 (4 units
  g of 128 = 2 heads each):
    qT  [A',T] = Wq'^T @ x (+bq', LARoPE)
    kT  [A',L] = Wk'^T @ ctx^T (+bk', LARoPE)
    v'  [L,8*65] = ctx @ Wv' (+bv'), with a ones-column per head
    attn[g,l,h2] [128,1024] = exp((k q^T)/32 + mask_bias)   (softmax
                numerator; logits are tiny so no max-subtraction)
    o_ps [65,512] = v'_h^T @ attn  (row 64 = denominator)
    (deferred normalization: denominators gathered into dn[16,512],
     one 1/x = exp(-log(x)) on ACT, PE ones-matmul broadcast, then
     osb = osb_u * bcast * x_mask)
    y [D,T] (f32, partial) = Wo'^T @ osb
  Host: out[b] = y_{b,0} + y_{b,1} + bo x xm.

All matmuls run in bf16 (fp32 PSUM accumulation).
"""

import contextlib
import math
import os
import sys

import numpy as np


def _ensure_paths():
    for p in ("/opt/trn_rl_repo", "/root/.axon_site/_ro/trn_rl_repo"):
        if os.path.isdir(p) and p not in sys.path:
            sys.path.insert(0, p)


try:
    import concourse.bass as bass  # noqa: F401
except ImportError:
    _ensure_paths()

import ml_dtypes
import concourse.bass as bass
import concourse.tile as tile
from concourse import bacc, bass2jax, mybir

# Problem shapes (hardcoded per the module definition).
B = 4
T = 1024
L = 1024
DM = 1024   # d_model
AD = 1024   # full attn_dim
ADH = 512   # per-core attn-dim slice (8 heads)
H = 16
HD = 64     # head dim
G = 4       # 128-wide units per core (2 heads each)
N_CORES = 8
SCALE = 1.0 / math.sqrt(AD)  # note: module scales by sqrt(attn_dim)
ROPE_GAMMA = 10.0
ROTARY_BASE = 10000.0
MASK_BIAS = -30000.0  # exp(x + MASK_BIAS) underflows to exactly 0.0 in fp32

MDT = mybir.dt.bfloat16
NP_MDT = ml_dtypes.bfloat16
F32 = mybir.dt.float32

AL = mybir.AluOpType
AF = mybir.ActivationFunctionType


def build_program():
    nc = bacc.Bacc("TRN2", target_bir_lowering=False, debug=False)

    def din(name, shape, dt):
        return nc.dram_tensor(name, shape, dt, kind="ExternalInput").ap()

    xs = din("xs", [DM, T], MDT)           # x[b]  (d_model x T)
    ctxT = din("ctxT", [DM, L], MDT)       # context[b].T
    wq = din("wq", [DM, ADH], MDT)         # Wq[:, a-slice]
    wk = din("wk", [DM, ADH], MDT)
    wv = din("wv", [DM, ADH], MDT)
    wo = din("wo", [ADH, DM], MDT)         # Wo[a-slice, :]
    bqc = din("bqc", [128, G], F32)        # bq'[g*128+p] at [p, g]
    bkc = din("bkc", [128, G], F32)
    bvr = din("bvr", [1, ADH], MDT)
    onesr = din("onesr", [1, 128], MDT)
    cosq = din("cosq", [128, T], F32)
    sin2q = din("sin2q", [128, T], F32)
    cosk = din("cosk", [128, L], F32)
    sin2k = din("sin2k", [128, L], F32)
    cmb = din("cmb", [128, 8], F32)        # key-mask bias per (p, l_tile)
    xmf16 = din("xmf16", [16, 512], F32)   # query mask, row r=g*4+th*2+h2
    perm = din("perm", [128, 128], MDT)    # partition permutation p -> p^32
    # bcast map per (g,th): out[p] = rx[4g+2th + p//64]  (lhsT slice gt*128)
    sel16 = din("sel16", [16, 8 * 128], MDT)
    y = nc.dram_tensor("y", [DM, T], F32, kind="ExternalOutput").ap()

    with tile.TileContext(nc) as tc, contextlib.ExitStack() as ctx:
        sb = ctx.enter_context(tc.tile_pool(name="sb", bufs=1))
        ps = ctx.enter_context(tc.tile_pool(name="ps", bufs=2, space="PSUM"))

        # ---- DMA, in consumption order ---------------------------------
        C = {}

        def cload(nm, ap):
            t = sb.tile(list(ap.shape), ap.dtype, tag=nm, name=f"c_{nm}", bufs=1)
            nc.sync.dma_start(t[:], ap)
            C[nm] = t

        # Q-phase needs
        cload("bqc", bqc)
        cload("cosq", cosq)
        cload("sin2q", sin2q)
        cload("perm", perm)
        cload("cmb", cmb)

        wq_t, xs_t = [], []
        xs_r = xs.rearrange("(n p) t -> n p t", p=128)
        wq_r = wq.rearrange("(n p) c -> n p c", p=128)
        for d in range(8):
            t = sb.tile([128, ADH], MDT, tag="w", bufs=24, name=f"wq{d}")
            nc.sync.dma_start(t[:], wq_r[d])
            wq_t.append(t)
            t = sb.tile([128, T], MDT, tag="xs", bufs=8, name=f"xs{d}")
            nc.sync.dma_start(t[:], xs_r[d])
            xs_t.append(t)

        # kproj(0) needs
        cload("bkc", bkc)
        cload("cosk", cosk)
        cload("sin2k", sin2k)
        ctx_t, wk_t = [], []
        ctx_r = ctxT.rearrange("(n p) l -> n p l", p=128)
        wk_r = wk.rearrange("(n p) c -> n p c", p=128)
        for d in range(8):
            t = sb.tile([128, L], MDT, tag="ctx", bufs=8, name=f"ctx{d}")
            nc.sync.dma_start(t[:], ctx_r[d])
            ctx_t.append(t)
            t = sb.tile([128, ADH], MDT, tag="w", bufs=24, name=f"wk{d}")
            nc.sync.dma_start(t[:], wk_r[d])
            wk_t.append(t)

        # V phase needs
        cload("bvr", bvr)
        cload("onesr", onesr)
        wv_t = []
        wv_r = wv.rearrange("(n p) c -> n p c", p=128)
        for d in range(8):
            t = sb.tile([128, ADH], MDT, tag="w", bufs=24, name=f"wv{d}")
            nc.sync.dma_start(t[:], wv_r[d])
            wv_t.append(t)

        # tail needs
        cload("xmf16", xmf16)
        cload("sel16", sel16)
        wo_t = []
        wo_r = wo.rearrange("(n p) c -> n p c", p=128)
        for a in range(4):
            t = sb.tile([128, DM], MDT, tag="wo", bufs=4, name=f"wo{a}")
            nc.sync.dma_start(t[:], wo_r[a])
            wo_t.append(t)

        # ---- persistent tiles ------------------------------------------
        qT_t = [sb.tile([128, T], MDT, tag="qT", bufs=4, name=f"qT{g}")
                for g in range(G)]
        kT_t = [sb.tile([128, L], MDT, tag="kT", bufs=4, name=f"kT{g}")
                for g in range(G)]
        vP_t = [None] * 8
        attn_t = [[[None] * 2 for _ in range(8)] for _ in range(G)]
        osb_u = [sb.tile([128, T], F32, tag="osbu", bufs=4, name=f"osbu{g}")
                 for g in range(G)]
        # denominators: DVE can only write partition bases {0,32,64,96}, so
        # stage each row on partition 0 and DMA-scatter onto 16 partitions
        # for one batched 1/x
        dn16 = sb.tile([16, 512], F32, tag="dn16", bufs=1, name="dn16")

        # ---- Q phase: qT[g] = rope(Wq'^T @ x + bq') --------------------
        # The perm-matmul of a unit is emitted with the NEXT unit's
        # projection so the PE never waits on the DVE stt results.
        pend_q = []

        def q_unit(g, th):
            tsl = slice(th * 512, (th + 1) * 512)
            q_ps = ps.tile([128, 512], F32, tag="pp", bufs=4, name=f"qps{g}_{th}")
            for d in range(8):
                nc.tensor.matmul(
                    q_ps[:], wq_t[d][:, g * 128:(g + 1) * 128], xs_t[d][:, tsl],
                    start=(d == 0), stop=(d == 7),
                )
            wsb = sb.tile([128, 512], MDT, tag="ropeW", bufs=3, name=f"qw{g}_{th}")
            nc.vector.scalar_tensor_tensor(
                wsb[:], q_ps[:], C["bqc"][:, g:g + 1], C["sin2q"][:, tsl],
                op0=AL.add, op1=AL.mult,
            )
            asb = sb.tile([128, 512], MDT, tag="ropeA", bufs=3, name=f"qa{g}_{th}")
            nc.vector.scalar_tensor_tensor(
                asb[:], q_ps[:], C["bqc"][:, g:g + 1], C["cosq"][:, tsl],
                op0=AL.add, op1=AL.mult,
            )
            pend_q.append((g, th, wsb, asb))

        def q_flush():
            g, th, wsb, asb = pend_q.pop(0)
            tsl = slice(th * 512, (th + 1) * 512)
            pw_ps = ps.tile([128, 512], F32, tag="pp", bufs=4, name=f"qpw{g}_{th}")
            nc.tensor.matmul(pw_ps[:], C["perm"][:], wsb[:], start=True, stop=True)
            nc.vector.tensor_add(qT_t[g][:, tsl], pw_ps[:], asb[:])

        # ---- V phase unit: v'[l] = (ctx @ Wv' + bv' | ones) ------------
        def v_unit(l):
            v_ps = ps.tile([128, 512], F32, tag="pp", bufs=4, name=f"vps{l}")
            for d in range(8):
                nc.tensor.matmul(
                    v_ps[:], ctx_t[d][:, l * 128:(l + 1) * 128], wv_t[d][:],
                    start=(d == 0), stop=False,
                )
            nc.tensor.matmul(
                v_ps[:], C["onesr"][0:1, 0:128], C["bvr"][0:1, :],
                start=False, stop=True,
            )
            vt = sb.tile([128, 520], MDT, tag="vP", bufs=8, name=f"vP{l}")
            out_ap = vt[:, :].rearrange("p (h e) -> p h e", e=65)[:, :, 0:64]
            in_ap = v_ps[:].rearrange("p (h d) -> p h d", d=64)
            nc.vector.tensor_copy(out_ap, in_ap)
            ones_ap = vt[:, :].rearrange("p (h e) -> p h e", e=65)[:, :, 64:65]
            nc.gpsimd.memset(ones_ap, 1.0)
            vP_t[l] = vt

        # ---- kproj(g) halves: kT[g] = rope(Wk'^T @ ctx^T + bk') --------
        kpend = {}

        def kproj_half(g, lh):
            if lh == 0:
                kpend[g] = []
            sl = slice(lh * 512, (lh + 1) * 512)
            k_ps = ps.tile([128, 512], F32, tag="pp", bufs=4, name=f"kps{g}_{lh}")
            for d in range(8):
                nc.tensor.matmul(
                    k_ps[:], wk_t[d][:, g * 128:(g + 1) * 128], ctx_t[d][:, sl],
                    start=(d == 0), stop=(d == 7),
                )
            wsb = sb.tile([128, 512], MDT, tag="ropeW", bufs=3, name=f"kw{g}_{lh}")
            nc.vector.scalar_tensor_tensor(
                wsb[:], k_ps[:], C["bkc"][:, g:g + 1], C["sin2k"][:, sl],
                op0=AL.add, op1=AL.mult,
            )
            asb = sb.tile([128, 512], MDT, tag="ropeA", bufs=3, name=f"ka{g}_{lh}")
            nc.vector.scalar_tensor_tensor(
                asb[:], k_ps[:], C["bkc"][:, g:g + 1], C["cosk"][:, sl],
                op0=AL.add, op1=AL.mult,
            )
            kpend[g].append((lh, wsb, asb))

        def kproj_flush(g):
            lh, wsb, asb = kpend[g].pop(0)
            sl = slice(lh * 512, (lh + 1) * 512)
            pw_ps = ps.tile([128, 512], F32, tag="pp", bufs=4, name=f"kpw{g}_{lh}")
            nc.tensor.matmul(pw_ps[:], C["perm"][:], wsb[:], start=True, stop=True)
            nc.vector.tensor_add(kT_t[g][:, sl], pw_ps[:], asb[:])

        # ---- QK + exp chunk: attn[g][l][h2] ----------------------------
        def qk_unit(g, l):
            qk0 = ps.tile([128, 1024], F32, tag="qk", bufs=2, name=f"qk{g}_{l}_0")
            qk1 = ps.tile([128, 1024], F32, tag="qk", bufs=2, name=f"qk{g}_{l}_1")
            lsl = slice(l * 128, (l + 1) * 128)
            for th in range(2):
                tsl = slice(th * 512, (th + 1) * 512)
                nc.tensor.matmul(
                    qk0[:, tsl], kT_t[g][0:64, lsl], qT_t[g][0:64, tsl],
                    start=True, stop=True,
                )
                nc.tensor.matmul(
                    qk1[:, tsl], kT_t[g][64:128, lsl], qT_t[g][64:128, tsl],
                    start=True, stop=True,
                )
            for h2, qk in ((0, qk0), (1, qk1)):
                at = sb.tile([128, 1024], MDT, tag="attn", bufs=22,
                             name=f"at{g}_{l}_{h2}")
                nc.scalar.activation(
                    at[:], qk[:], AF.Exp, bias=C["cmb"][:, l:l + 1], scale=SCALE,
                )
                attn_t[g][l][h2] = at

        # ---- PV chunk: o(g,h2,th) + unnormalized evacuation ------------
        def pv_unit(g, h2, th):
            h = 2 * g + h2
            tsl = slice(th * 512, (th + 1) * 512)
            o_ps = ps.tile([65, 512], F32, tag="pp", bufs=4, name=f"o{g}_{h2}_{th}")
            for l in range(8):
                nc.tensor.matmul(
                    o_ps[:], vP_t[l][:, h * 65:h * 65 + 65],
                    attn_t[g][l][h2][:, tsl],
                    start=(l == 0), stop=(l == 7),
                )
            nc.vector.tensor_copy(
                osb_u[g][h2 * 64:(h2 + 1) * 64, tsl], o_ps[0:64, :])
            r = 4 * g + 2 * th + h2
            stg = sb.tile([1, 512], F32, tag="dnstage", bufs=2, name=f"dnst{r}")
            nc.vector.tensor_copy(stg[:], o_ps[64:65, :])
            nc.sync.dma_start(dn16[r:r + 1, :], stg[:])

        # ---- schedule --------------------------------------------------
        def interleave(qks, others):
            # spread `others` between QK chunks so the PE never waits
            # out a full exp
            n = max(len(qks), len(others))
            for j in range(n):
                if j < len(qks):
                    qks[j]()
                if j < len(others):
                    others[j]()

        # prologue: Q(0), Q(1), kproj(0)
        q_unit(0, 0)
        q_unit(0, 1)
        q_flush()
        q_unit(1, 0)
        q_flush()
        q_unit(1, 1)
        q_flush()
        kproj_half(0, 0)
        q_flush()
        kproj_half(0, 1)
        kproj_flush(0)
        kproj_flush(0)

        # steps
        for s in range(5):
            qks = ([lambda l=l, g=s: qk_unit(g, l) for l in range(8)]
                   if s < 4 else [])
            others = []
            if s == 0:
                for l in range(8):
                    others.append(lambda l=l: v_unit(l))
                others.append(lambda: kproj_half(1, 0))
                others.append(lambda: (kproj_half(1, 1), kproj_flush(1)))
                others.append(lambda: kproj_flush(1))
                others.append(lambda: q_unit(2, 0))
                others.append(lambda: (q_flush(), q_unit(2, 1)))
                others.append(lambda: q_flush())
            elif s == 1:
                others.append(lambda: kproj_half(2, 0))
                others.append(lambda: (kproj_half(2, 1), kproj_flush(2)))
                others.append(lambda: kproj_flush(2))
                others.append(lambda: q_unit(3, 0))
                others.append(lambda: (q_flush(), q_unit(3, 1)))
                others.append(lambda: q_flush())
                for h2 in range(2):
                    for th in range(2):
                        others.append(lambda h2=h2, th=th: pv_unit(0, h2, th))
            elif s == 2:
                others.append(lambda: kproj_half(3, 0))
                others.append(lambda: (kproj_half(3, 1), kproj_flush(3)))
                others.append(lambda: kproj_flush(3))
                for h2 in range(2):
                    for th in range(2):
                        others.append(lambda h2=h2, th=th: pv_unit(1, h2, th))
            elif s == 3:
                for h2 in range(2):
                    for th in range(2):
                        others.append(lambda h2=h2, th=th: pv_unit(2, h2, th))
            else:
                for h2 in range(2):
                    for th in range(2):
                        others.append(lambda h2=h2, th=th: pv_unit(3, h2, th))
            interleave(qks, others)

        # ---- tail: normalization + out-projection ----------------------
        # 1/dn = exp(-log(dn)) on ACT (both fns share a table set; DVE's
        # iterative reciprocal would cost 4.3us on this shape)
        lnv = sb.tile([16, 512], F32, tag="lnv", bufs=1, name="lnv")
        nc.scalar.activation(lnv[:], dn16[:], AF.Ln)
        rcp = sb.tile([16, 512], F32, tag="rcp", bufs=1, name="rcp")
        nc.scalar.activation(rcp[:], lnv[:], AF.Exp, scale=-1.0)
        rx = sb.tile([16, 512], MDT, tag="rx", bufs=1, name="rx")
        nc.vector.tensor_mul(rx[:], rcp[:], C["xmf16"][:])

        osb_t = [sb.tile([128, T], MDT, tag="osb", bufs=4, name=f"osb{g}")
                 for g in range(G)]

        def norm_mul(g, th):
            tsl = slice(th * 512, (th + 1) * 512)
            gt = 2 * g + th
            bc_ps = ps.tile([128, 512], F32, tag="pp", bufs=4, name=f"bc{g}_{th}")
            nc.tensor.matmul(bc_ps[:], C["sel16"][:, gt * 128:(gt + 1) * 128],
                             rx[:], start=True, stop=True)
            nc.vector.tensor_mul(osb_t[g][:, tsl], osb_u[g][:, tsl], bc_ps[:])

        y_r = y.rearrange("(n p) t -> n p t", p=128)

        def oproj(d, th):
            tsl = slice(th * 512, (th + 1) * 512)
            o_ps = ps.tile([128, 512], F32, tag="pp", bufs=4, name=f"ops{d}_{th}")
            for a in range(4):
                nc.tensor.matmul(
                    o_ps[:], wo_t[a][:, d * 128:(d + 1) * 128],
                    osb_t[a][:, tsl], start=(a == 0), stop=(a == 3),
                )
            yt = sb.tile([128, 512], F32, tag="outsb", bufs=4, name=f"yt{d}_{th}")
            nc.scalar.copy(yt[:], o_ps[:])  # ACT; keeps DVE free
            nc.sync.dma_start(y_r[d][:, tsl], yt[:])

        for g in range(G):
            norm_mul(g, 0)
        for d in range(8):
            if d < 4:
                norm_mul(d, 1)
            oproj(d, 0)
        for d in range(8):
            oproj(d, 1)

    nc.compile()
    return nc


_PROGRAM = None


def _get_program():
    global _PROGRAM
    if _PROGRAM is None:
        _PROGRAM = build_program()
    return _PROGRAM


def _host_prep(x, context, x_mask, context_mask, Wq, bq, Wk, bk, Wv, bv, Wo, bo):
    """Build the 8 per-core input maps."""
    f32 = np.float32
    x = np.asarray(x, f32)
    context = np.asarray(context, f32)
    xm = np.asarray(x_mask).astype(f32)        # [B,1,T]
    cm = np.asarray(context_mask).astype(f32)  # [B,1,L]

    len_q = xm.sum(axis=(1, 2))  # [B]
    len_k = cm.sum(axis=(1, 2))

    inv_freq = 1.0 / (ROTARY_BASE ** (np.arange(0, HD, 2, dtype=f32) / HD))
    theta = (inv_freq * ROPE_GAMMA).astype(f32)  # [32]

    p = np.arange(128)
    pm32 = p % 32
    sgn_sin2 = np.where((p % 64) < 32, 1.0, -1.0).astype(f32)[:, None]

    perm = np.zeros((128, 128), f32)
    perm[p, p ^ 32] = 1.0  # lhsT: out[m] = sum_k perm[k, m] * in[k] = in[m^32]

    sel16 = np.zeros((16, 8 * 128), f32)
    for g in range(G):
        for th in range(2):
            gt = 2 * g + th
            sel16[4 * g + 2 * th + 0, gt * 128:gt * 128 + 64] = 1.0
            sel16[4 * g + 2 * th + 1, gt * 128 + 64:(gt + 1) * 128] = 1.0

    Wq = np.asarray(Wq, f32)
    Wk = np.asarray(Wk, f32)
    Wv = np.asarray(Wv, f32)
    Wo = np.asarray(Wo, f32)
    bq = np.asarray(bq, f32)
    bk = np.asarray(bk, f32)
    bv = np.asarray(bv, f32)

    in_maps = []
    for c in range(N_CORES):
        b, hh = c // 2, c % 2
        asl = slice(hh * ADH, (hh + 1) * ADH)

        pos_q = np.arange(T, dtype=f32) / len_q[b]
        fr_q = pos_q[None, :] * theta[pm32][:, None]       # [128, T]
        pos_k = np.arange(L, dtype=f32) / len_k[b]
        fr_k = pos_k[None, :] * theta[pm32][:, None]       # [128, L]

        m = {}
        m["xs"] = np.ascontiguousarray(x[b]).astype(NP_MDT)
        m["ctxT"] = np.ascontiguousarray(context[b].T).astype(NP_MDT)
        m["wq"] = np.ascontiguousarray(Wq[:, asl]).astype(NP_MDT)
        m["wk"] = np.ascontiguousarray(Wk[:, asl]).astype(NP_MDT)
        m["wv"] = np.ascontiguousarray(Wv[:, asl]).astype(NP_MDT)
        m["wo"] = np.ascontiguousarray(Wo[asl, :]).astype(NP_MDT)
        m["bqc"] = bq[asl].reshape(G, 128).T.copy()
        m["bkc"] = bk[asl].reshape(G, 128).T.copy()
        m["bvr"] = bv[asl].reshape(1, ADH).astype(NP_MDT)
        m["onesr"] = np.ones((1, 128), NP_MDT)
        m["perm"] = perm.astype(NP_MDT)
        m["sel16"] = sel16.astype(NP_MDT)
        m["cosq"] = np.cos(fr_q).astype(f32)
        m["sin2q"] = (np.sin(fr_q) * sgn_sin2).astype(f32)
        m["cosk"] = np.cos(fr_k).astype(f32)
        m["sin2k"] = (np.sin(fr_k) * sgn_sin2).astype(f32)
        # 0.0 where the key is valid, MASK_BIAS where masked
        m["cmb"] = ((cm[b, 0] - 1.0) * (-MASK_BIAS)).reshape(8, 128).T.copy().astype(f32)
        xmf16 = np.empty((16, 512), f32)
        for g in range(G):
            for th in range(2):
                for h2 in range(2):
                    xmf16[4 * g + 2 * th + h2] = xm[b, 0, th * 512:(th + 1) * 512]
        m["xmf16"] = xmf16
        in_maps.append(m)
    return in_maps


_JIT_CACHE = {}


def _run_concurrent(nc, in_maps, n_cores=N_CORES):
    """Run the same bass program on n_cores devices concurrently, one
    single-device PJRT execute per core.

    (run_bass_kernel_spmd's multi-core path uses an 8-device shard_map
    SPMD executable, which hangs under this axon terminal; per-device
    dispatch of the identical program is functionally equivalent for a
    collective-free kernel and works.)
    """
    import jax

    bass2jax.install_neuronx_cc_hook()
    key = id(nc)
    if key not in _JIT_CACHE:
        partition_name = (
            nc.partition_id_tensor.name if nc.partition_id_tensor else None
        )
        in_names, out_names, out_avals, zero_outs = [], [], [], []
        for alloc in nc.m.functions[0].allocations:
            if not isinstance(alloc, mybir.MemoryLocationSet):
                continue
            name = alloc.memorylocations[0].name
            if alloc.kind == "ExternalInput":
                if name != partition_name:
                    in_names.append(name)
            elif alloc.kind == "ExternalOutput":
                shape = tuple(alloc.tensor_shape)
                dtype = mybir.dt.np(alloc.dtype)
                out_names.append(name)
                out_avals.append(jax.core.ShapedArray(shape, dtype))
                zero_outs.append(np.zeros(shape, dtype))
        n_params = len(in_names)
        in_names_full = list(in_names) + list(out_names)
        if partition_name is not None:
            in_names_full.append(partition_name)
        in_names_full = tuple(in_names_full)

        def _body(*args):
            operands = list(args)
            if partition_name is not None:
                operands.append(bass2jax.partition_id_tensor())
            outs = bass2jax._bass_exec_p.bind(
                *operands, out_avals=tuple(out_avals), in_names=in_names_full,
                out_names=tuple(out_names), lowering_input_output_aliases=(),
                sim_require_finite=True, sim_require_nnan=True, nc=nc)
            return tuple(outs)

        donate_idx = tuple(range(n_params, n_params + len(out_names)))
        jfn = jax.jit(_body, donate_argnums=donate_idx, keep_unused=True)
        _JIT_CACHE[key] = (jfn, in_names, out_names, zero_outs)

    jfn, in_names, out_names, zero_outs = _JIT_CACHE[key]
    devices = jax.devices()[:n_cores]
    futs = []
    for c, dev in enumerate(devices):
        args = [jax.device_put(np.asarray(in_maps[c][nm]), dev) for nm in in_names]
        args += [jax.device_put(z, dev) for z in zero_outs]
        futs.append(jfn(*args))
    return [
        {nm: np.asarray(futs[c][i]) for i, nm in enumerate(out_names)}
        for c in range(n_cores)
    ]


def precompile():
    """AOT-compile the NEFF (client-side) without touching the data plane."""
    import jax

    nc = _get_program()
    _run_concurrent(nc, [], n_cores=0)  # populate _JIT_CACHE only
    jfn, in_names, out_names, zero_outs = _JIT_CACHE[id(nc)]
    specs = []
    for alloc in nc.m.functions[0].allocations:
        if not isinstance(alloc, mybir.MemoryLocationSet):
            continue
        name = alloc.memorylocations[0].name
        if alloc.kind == "ExternalInput" and name in in_names:
            specs.append((name, jax.ShapeDtypeStruct(
                tuple(alloc.tensor_shape), mybir.dt.np(alloc.dtype))))
    by_name = dict(specs)
    args = [by_name[nm] for nm in in_names]
    args += [jax.ShapeDtypeStruct(z.shape, z.dtype) for z in zero_outs]
    compiled = jfn.lower(*args).compile()
    return compiled


def kernel(x, context, x_mask, context_mask, Wq, bq, Wk, bk, Wv, bv, Wo, bo):
    nc = _get_program()
    in_maps = _host_prep(x, context, x_mask, context_mask,
                         Wq, bq, Wk, bk, Wv, bv, Wo, bo)
    results = _run_concurrent(nc, in_maps, N_CORES)

    xm = np.asarray(x_mask).astype(np.float32)  # [B,1,T]
    bo = np.asarray(bo, np.float32)
    out = np.empty((B, DM, T), np.float32)
    for b in range(B):
        out[b] = results[2 * b]["y"] + results[2 * b + 1]["y"]
        out[b] += bo[:, None] * xm[b, 0][None, :]
    return out
